# revision 3
# baseline (speedup 1.0000x reference)
"""Causal attention (single head, d=1024) on 8 trn2 NeuronCores — v2.

Sharding: data-parallel over batch (4) x 2-way interleaved query split per
batch (core c: batch c//2, query blocks {h, h+2, ..., h+14}, h=c%2).

Algebraic restructure vs v1 (all per core, q = its 1024 query rows):
  W_qk = W_q^T @ W_k  (host, f64 -> bf16): scores = x_q W_qk x_k^T, so the
  K projection disappears (keys are raw x), and
  out = P @ x @ W_v^T = (P @ x) @ W_v^T, so the V projection runs on the
  1024 OWN rows' reduction result instead of all 2048 tokens (removes the
  cross-core duplicated full-token V projection).

Device pipeline (single NEFF, SPMD, all matmuls bf16 except noted):
  A: GT[e,q]    = W_qk^T x_q^T          (128 mm, N=512)
  B: kb-major scores^T[k,q] = x_k GT    (N up to 1024), additive causal
     mask on the single diagonal 128-block, exp (ACT) -> Pt bf16 in SBUF
  C: per j: U[q,d] = Pt^T x_row (N=512), row-sums l via ones column (N=1),
     PE-transpose U (f32r), out = U W_v^T (N=512), DMA out + l.
Host: W_qk precompute, row normalization by l, tril(k=1) single leak
element patched exactly (as v1), un-permutation of query blocks.
"""

import numpy as np
import ml_dtypes

import concourse.bass as bass
import concourse.mybir as mybir
import concourse.tile as tile
from concourse import bacc
from concourse.masks import make_identity
from concourse.bass_utils import run_bass_kernel_spmd

B, T, D = 4, 2048, 1024
NCORES = 8
NQB = 8            # query blocks per core (128 rows each)
KB = 16            # key blocks
NEG = -1.0e9
SCALE = 1.0 / 32.0  # 1/sqrt(1024)

F32 = mybir.dt.float32
F32R = mybir.dt.float32r
BF16 = mybir.dt.bfloat16
BF16NP = ml_dtypes.bfloat16

LAST_RESULT = None


def _build(repeat=None):
    nc = bacc.Bacc(None, target_bir_lowering=False)

    xT_d = nc.dram_tensor("xT", [D, T], BF16, kind="ExternalInput")
    xrow_d = nc.dram_tensor("xrow", [T, D], BF16, kind="ExternalInput")
    xq_d = nc.dram_tensor("xq", [D, D], BF16, kind="ExternalInput")
    wqk_d = nc.dram_tensor("wqk", [D, D], BF16, kind="ExternalInput")
    wvT_d = nc.dram_tensor("wvT", [D, D], BF16, kind="ExternalInput")
    masks_d = nc.dram_tensor("masks", [128, KB, 128], F32, kind="ExternalInput")
    ones_d = nc.dram_tensor("ones", [128, 1], BF16, kind="ExternalInput")
    out_d = nc.dram_tensor("out", [D, D], F32, kind="ExternalOutput")
    l_d = nc.dram_tensor("lsum", [128, NQB], F32, kind="ExternalOutput")

    with tile.TileContext(nc) as tc:
        with tc.tile_pool(name="persist", bufs=1) as persist:
            wqk = persist.tile([128, 8, D], BF16, tag="wqk")
            wvT = persist.tile([128, 8, D], BF16, tag="wvT")
            masks_s = persist.tile([128, KB, 128], F32, tag="masks")
            ones_s = persist.tile([128, 1], BF16, tag="ones")
            ident_f = persist.tile([128, 128], F32, tag="ident_f")
            ident = persist.tile([128, 128], F32R, tag="ident")

            for i in range(8):
                nc.scalar.dma_start(out=wqk[:, i, :],
                                    in_=wqk_d[i * 128:(i + 1) * 128, :])
                nc.scalar.dma_start(out=wvT[:, i, :],
                                    in_=wvT_d[i * 128:(i + 1) * 128, :])
            nc.scalar.dma_start(out=masks_s[:, :, :], in_=masks_d[:, :, :])
            nc.scalar.dma_start(out=ones_s[:, :], in_=ones_d[:, :])
            make_identity(nc, ident_f)
            nc.vector.tensor_copy(ident, ident_f)

            import contextlib
            loop_ctx = (
                tc.For_i(0, repeat, 1) if repeat else contextlib.nullcontext()
            )
            with loop_ctx:
                _body(nc, tc, wqk, wvT, masks_s, ones_s, ident,
                      xT_d, xrow_d, xq_d, out_d, l_d)

    nc.compile()
    return nc


def _body(nc, tc, wqk, wvT, masks_s, ones_s, ident,
          xT_d, xrow_d, xq_d, out_d, l_d):
    with tc.tile_pool(name="xb", bufs=1) as px:
        xT = px.tile([128, 8, T], BF16, tag="xT")
        xrow = px.tile([128, KB, D], BF16, tag="xrow")
        xq = px.tile([128, 8, D], BF16, tag="xq")
        GT = px.tile([128, 8, D], BF16, tag="GT")
        Pt = px.tile([128, KB, D], BF16, tag="Pt")
        lt = px.tile([128, NQB], F32, tag="lt")

        for i in range(8):
            nc.gpsimd.dma_start(out=xq[:, i, :],
                                in_=xq_d[i * 128:(i + 1) * 128, :])
        for i in range(8):
            nc.gpsimd.dma_start(out=xT[:, i, :],
                                in_=xT_d[i * 128:(i + 1) * 128, :])
        for kb in range(KB):
            nc.gpsimd.dma_start(out=xrow[:, kb, :],
                                in_=xrow_d[kb * 128:(kb + 1) * 128, :])

        # ---------------- A: GT[e, q] = (x_q @ W_qk)^T ----------------
        with tc.tile_pool(name="psA", bufs=2, space="PSUM") as psA:
            for eb in range(8):
                ps = psA.tile([128, D], F32, tag="g")
                for i in range(8):
                    for ch in range(2):
                        nc.tensor.matmul(
                            ps[:, ch * 512:(ch + 1) * 512],
                            lhsT=wqk[:, i, eb * 128:(eb + 1) * 128],
                            rhs=xq[:, i, ch * 512:(ch + 1) * 512],
                            start=(i == 0),
                            stop=(i == 7),
                        )
                nc.vector.tensor_copy(GT[:, eb, :], ps)

        # ---------------- B: scores^T + exp, kb-major ----------------
        with tc.tile_pool(name="psB", bufs=3, space="PSUM") as psB:
            for kb in range(KB):
                j0 = kb // 2
                ncols = (NQB - j0) * 128
                ps = psB.tile([128, 1024], F32, tag="s")
                for eb in range(8):
                    for cs in range(0, ncols, 512):
                        ce = min(cs + 512, ncols)
                        nc.tensor.matmul(
                            ps[:, cs:ce],
                            lhsT=xT[:, eb, kb * 128:(kb + 1) * 128],
                            rhs=GT[:, eb, j0 * 128 + cs:j0 * 128 + ce],
                            start=(eb == 0),
                            stop=(eb == 7),
                        )
                # additive causal mask on the first eligible (diagonal) block
                nc.vector.scalar_tensor_tensor(
                    out=ps[:, 0:128],
                    in0=ps[:, 0:128],
                    scalar=1.0,
                    in1=masks_s[:, kb, :],
                    op0=mybir.AluOpType.mult,
                    op1=mybir.AluOpType.add,
                )
                nc.scalar.activation(
                    out=Pt[:, kb, j0 * 128:j0 * 128 + ncols],
                    in_=ps[:, 0:ncols],
                    func=mybir.ActivationFunctionType.Exp,
                    scale=SCALE,
                )

        # ---------------- C: per-j U = P@x, l, U^T, out = U@W_v^T ------
        with (
            tc.tile_pool(name="attc", bufs=1) as ac,
            tc.tile_pool(name="psU", bufs=3, space="PSUM") as psU,
            tc.tile_pool(name="psL", bufs=1, space="PSUM") as psL,
            tc.tile_pool(name="psT", bufs=2, space="PSUM") as psT,
            tc.tile_pool(name="psO", bufs=1, space="PSUM") as psO,
        ):
            for j in range(NQB):
                cap = 2 * j + 2
                psu = [psU.tile([128, 512], F32, tag="u", name=f"psu{oc}")
                       for oc in range(2)]
                psl = psL.tile([128, 1], F32, tag="l")
                for kb in range(cap):
                    lhsTp = Pt[:, kb, j * 128:(j + 1) * 128]
                    for oc in range(2):
                        nc.tensor.matmul(
                            psu[oc],
                            lhsT=lhsTp,
                            rhs=xrow[:, kb, oc * 512:(oc + 1) * 512],
                            start=(kb == 0),
                            stop=(kb == cap - 1),
                        )
                    nc.tensor.matmul(
                        psl,
                        lhsT=lhsTp,
                        rhs=ones_s,
                        start=(kb == 0),
                        stop=(kb == cap - 1),
                    )
                U_sb = ac.tile([128, D], F32R, tag="usb", bufs=2)
                for oc in range(2):
                    nc.vector.tensor_copy(
                        U_sb[:, oc * 512:(oc + 1) * 512], psu[oc]
                    )
                nc.vector.tensor_copy(lt[:, j:j + 1], psl)
                UT = ac.tile([128, 8, 128], BF16, tag="ut", bufs=2)
                for i in range(8):
                    pst = psT.tile([128, 128], F32R, tag="t")
                    nc.tensor.transpose(
                        pst, U_sb[:, i * 128:(i + 1) * 128], ident
                    )
                    nc.vector.tensor_copy(UT[:, i, :], pst)
                pso = psO.tile([128, D], F32, tag="o")
                for i in range(8):
                    for oc in range(2):
                        nc.tensor.matmul(
                            pso[:, oc * 512:(oc + 1) * 512],
                            lhsT=UT[:, i, :],
                            rhs=wvT[:, i, oc * 512:(oc + 1) * 512],
                            start=(i == 0),
                            stop=(i == 7),
                        )
                outs = ac.tile([128, D], F32, tag="os", bufs=2)
                nc.scalar.copy(outs, pso)
                nc.sync.dma_start(
                    out=out_d[j * 128:(j + 1) * 128, :], in_=outs
                )
            nc.sync.dma_start(out=l_d[:, :], in_=lt)


_NC = None


def _get_nc():
    global _NC
    if _NC is None:
        _NC = _build()
    return _NC


def _qrows(h):
    return np.concatenate(
        [np.arange(128 * (2 * j + h), 128 * (2 * j + h) + 128)
         for j in range(NQB)]
    )


def _host_masks(h):
    m = np.zeros((128, KB, 128), dtype=np.float32)
    kp = np.arange(128)[:, None]
    r = np.arange(128)[None, :]
    for kb in range(KB):
        qb = 2 * (kb // 2) + h
        kglob = 128 * kb + kp
        qglob = 128 * qb + r
        # leak key 128*(qb+1) is patched on the host, so clip at the
        # diag-block boundary in addition to the tril(k=1) rule
        vis = (kglob <= qglob + 1) & (kglob < 128 * (qb + 1))
        m[:, kb, :] = np.where(vis, 0.0, NEG)
    return m


def _in_maps(x, W_q, W_k, W_v):
    x = np.asarray(x, dtype=np.float32)
    wqk = (W_q.astype(np.float64).T @ W_k.astype(np.float64)).astype(BF16NP)
    wvT = np.ascontiguousarray(W_v.astype(np.float32).T).astype(BF16NP)
    masks_h = [_host_masks(0), _host_masks(1)]
    ones = np.ones((128, 1), dtype=BF16NP)

    in_maps = []
    for c in range(NCORES):
        b, h = c // 2, c % 2
        xb = x[b]
        in_maps.append({
            "xT": np.ascontiguousarray(xb.T).astype(BF16NP),
            "xrow": xb.astype(BF16NP),
            "xq": np.ascontiguousarray(xb[_qrows(h)].T).astype(BF16NP),
            "wqk": wqk,
            "wvT": wvT,
            "masks": masks_h[h],
            "ones": ones,
        })
    return in_maps


def kernel(x, W_q, W_k, W_v):
    x = np.asarray(x, dtype=np.float32)
    W_q = np.asarray(W_q, dtype=np.float32)
    W_k = np.asarray(W_k, dtype=np.float32)
    W_v = np.asarray(W_v, dtype=np.float32)

    nc = _get_nc()
    in_maps = _in_maps(x, W_q, W_k, W_v)

    global LAST_RESULT
    res = run_bass_kernel_spmd(nc, in_maps, core_ids=list(range(NCORES)))
    LAST_RESULT = res

    out = np.empty((B, T, D), dtype=np.float32)
    for c in range(NCORES):
        b, h = c // 2, c % 2
        o = res.results[c]["out"].astype(np.float64)
        l = res.results[c]["lsum"].astype(np.float64)
        for j in range(NQB):
            qb = 2 * j + h
            ltot = l[:, j].copy()
            rows = o[j * 128:(j + 1) * 128, :]
            kglob = 128 * (qb + 1)
            if kglob < T:
                # tril(k=1): row 127 of this block also sees key `kglob`,
                # which the device skipped — patch that single element here.
                qrow = x[b, 128 * qb + 127].astype(np.float64)
                xk = x[b, kglob].astype(np.float64)
                krow = W_k.astype(np.float64) @ xk
                vrow = W_v.astype(np.float64) @ xk
                p = np.exp((qrow @ W_q.T.astype(np.float64)) @ krow / 32.0)
                rows[127, :] = rows[127, :] + p * vrow
                ltot[127] = ltot[127] + p
            out[b, 128 * qb:128 * (qb + 1), :] = (
                rows / ltot[:, None]
            ).astype(np.float32)
    return out
